# revision 35
# baseline (speedup 1.0000x reference)
"""Trainium2 Bass kernel for nn_AttentionUnit (multi-head attention block).

Reference math (B=2, S=2048, D=1024, H=16 heads, d_head=64, fp32):
    Q = q @ wq.T + bq ; K = k @ wk.T + bk ; V = v @ wv.T + bv
    S = QK^T / 8  (per head), causal mask + key-padding mask
    out = softmax(S) @ V  -> concat heads -> @ wo.T + bo

Sharding (8 cores): data-parallel over batch (2 groups of 4 cores),
tensor-parallel over heads (4 heads/core).  Column-parallel QKV,
row-parallel wo with per-q-block in-group ReduceScatter(add) of the
partial outputs, interleaved with the attention loop so the collectives
overlap compute; the host reassembles the chunk-major outputs.

Device-side layout choices:
  - All activations/weights enter as bf16 (fp32 accumulation in PSUM);
    host pre-transposes x to x^T[D, S] and appends a ones-row so
    projection biases ride the matmul contraction ([D+1] rows).
  - Q^T/K^T are kept head-major [65, S]: row 64 of Q^T is ones, row 64
    of K^T is -30000*key_padding_mask, so padded keys get exp()==0 via
    the same contraction trick.
  - Scores are computed transposed (S^T[k, q], keys on partitions) so
    exp(S^T) = P^T feeds P@V directly as the stationary operand;
    softmax max-subtraction is skipped (scores are O(1) here).  The
    row-sum rides a ones-column appended to V; ctx lands q-major so the
    division is a per-partition reciprocal+scale, then a PE transpose
    puts ctx^T in place for the row-parallel output projection.
  - Causal masking is exact: k-tiles are sliced to the valid q range,
    and the 4 diagonal-band tiles per 512-wide q block are multiplied
    by constant 0/1 masks after exp().
  - Input DMAs round-robin across the three DMA-capable queues
    (sync/scalar/gpsimd); collectives own the gpsimd queue in steady
    state.
"""

import os
import sys
from contextlib import ExitStack

import numpy as np

try:
    import concourse.bass as bass
except ImportError:  # harness containers keep the repo at /opt/trn_rl_repo
    sys.path.insert(0, "/opt/trn_rl_repo")
    import concourse.bass as bass

from concourse import bacc

import ml_dtypes
import concourse.mybir as mybir
import concourse.tile as tile
from concourse.bass_utils import run_bass_kernel_spmd

BF16 = ml_dtypes.bfloat16

B = 2
SEQ = 2048
D = 1024
H = 16
DH = 64
NCORES = 8
G = 4            # tensor-parallel group size (cores per batch)
HPC = H // G     # heads per core
DPC = HPC * DH   # head dims per core (256)
QB = 512         # q block width
KT = 128         # k tile height
NEG = -30000.0   # "minus infinity" for masking (exp() underflows to 0)


def build_program(seq=SEQ, d=D, hpc=HPC, trace_friendly=False):
    """Emit the SPMD program (identical on all 8 cores)."""
    fp32 = mybir.dt.float32
    bf16 = mybir.dt.bfloat16
    dpc = hpc * DH
    n_qb = seq // QB
    n_kt = seq // KT
    n_dt = d // 128          # 128-row tiles of the model dim
    n_mt = dpc // 128        # 128-col tiles of the per-core head dims
    sub = QB // KT           # k-tiles per q block on the diagonal (4)
    rsp = 1                           # q blocks per ReduceScatter chunk
    out_rows = seq // G

    nc = bacc.Bacc(num_devices=NCORES)

    xqT = nc.declare_dram_parameter("xqT", [d + 1, seq], bf16, False)
    xkT = nc.declare_dram_parameter("xkT", [d + 1, seq], bf16, False)
    xvT = nc.declare_dram_parameter("xvT", [d + 1, seq], bf16, False)
    wqT = nc.declare_dram_parameter("wqT", [d + 1, dpc], bf16, False)
    wkT = nc.declare_dram_parameter("wkT", [d + 1, dpc], bf16, False)
    wvT = nc.declare_dram_parameter("wvT", [d + 1, dpc], bf16, False)
    woT = nc.declare_dram_parameter("woT", [dpc, d], bf16, False)
    masks_d = nc.declare_dram_parameter("masks", [KT, sub, QB], bf16, False)
    ident_d = nc.declare_dram_parameter("ident", [128, 128], bf16, False)
    onesrow_d = nc.declare_dram_parameter("onesrow", [1, seq], bf16, False)
    padrow_d = nc.declare_dram_parameter("padrow", [1, seq], bf16, False)
    bo4_d = nc.declare_dram_parameter("bo4", [128, d], fp32, False)
    out_ext = nc.declare_dram_parameter("out", [n_qb // rsp, rsp * 128, d], bf16, isOutput=True)

    partial_dram = nc.dram_tensor("partial", [seq, d], bf16)
    rs_out = nc.dram_tensor("rs_out", [n_qb // rsp, rsp * 128, d], bf16)
    last_split = False  # two tail collectives serialize: worse
    rs_l = nc.dram_tensor("rs_l", [2, 64, d], bf16) if last_split else None

    groups = [[0, 1, 2, 3], [4, 5, 6, 7]]

    with ExitStack() as ctx:
        tc = ctx.enter_context(tile.TileContext(nc, num_cores=NCORES))

        xpool = ctx.enter_context(tc.tile_pool(name="xp", bufs=12))
        wpool = ctx.enter_context(tc.tile_pool(name="wp", bufs=12))
        persist = ctx.enter_context(tc.tile_pool(name="persist", bufs=1))
        vpool = ctx.enter_context(tc.tile_pool(name="vp", bufs=1))
        ppool = ctx.enter_context(tc.tile_pool(name="pp", bufs=20))
        spool = ctx.enter_context(tc.tile_pool(name="sp", bufs=4))
        opool = ctx.enter_context(tc.tile_pool(name="op", bufs=4))
        cqpool = ctx.enter_context(tc.tile_pool(name="cq", bufs=3))
        psA = ctx.enter_context(tc.tile_pool(name="psA", bufs=3, space="PSUM"))
        psB = ctx.enter_context(tc.tile_pool(name="psB", bufs=2, space="PSUM"))
        psT = ctx.enter_context(tc.tile_pool(name="psT", bufs=1, space="PSUM"))
        psD = ctx.enter_context(tc.tile_pool(name="psD", bufs=2, space="PSUM"))

        # ---- constants ----
        masks_sb = persist.tile([KT, sub, QB], bf16, tag="masks")
        nc.gpsimd.dma_start(out=masks_sb, in_=masks_d[:, :, :])
        bo4_sb = persist.tile([128, d], fp32, tag="bo4")
        nc.gpsimd.dma_start(out=bo4_sb, in_=bo4_d[:, :])
        ident_sb = persist.tile([128, 128], bf16, tag="ident")
        nc.gpsimd.dma_start(out=ident_sb, in_=ident_d[:, :])

        # ---- persistent activation tiles ----
        QT = [persist.tile([65, seq], bf16, tag=f"QT{h}", name=f"QT{h}") for h in range(hpc)]
        KTt = [persist.tile([65, seq], bf16, tag=f"KT{h}", name=f"KT{h}") for h in range(hpc)]
        V_sb = [vpool.tile([128, hpc, 65], bf16, tag=f"V{m}", name=f"V{m}") for m in range(n_kt)]
        ctxT = [persist.tile([128, seq], bf16, tag=f"ctxT{t}", name=f"ctxT{t}") for t in range(n_mt)]

        # ================= phase A: projections =================
        def qk_proj(xT_d, wT_d, dest, pad_d, evict_eng, dma_engs):
            xt = []
            wt = []
            for kti in range(n_dt):
                eng = dma_engs[kti % len(dma_engs)]
                x_t = xpool.tile([128, seq], bf16, tag="xt")
                eng.dma_start(out=x_t, in_=xT_d[kti * 128:(kti + 1) * 128, :])
                w_t = wpool.tile([128, dpc], bf16, tag="wt")
                eng.dma_start(out=w_t, in_=wT_d[kti * 128:(kti + 1) * 128, :])
                xt.append(x_t)
                wt.append(w_t)
            x_last = xpool.tile([1, seq], bf16, tag="xlast")
            dma_engs[0].dma_start(out=x_last, in_=xT_d[d:d + 1, :])
            w_last = wpool.tile([1, dpc], bf16, tag="wlast")
            dma_engs[0].dma_start(out=w_last, in_=wT_d[d:d + 1, :])
            xt.append(x_last)
            wt.append(w_last)

            for mt in range(n_mt):
                for nq in range(n_qb):
                    ps = psD.tile([128, QB], fp32, tag="proj")
                    for kti in range(n_dt + 1):
                        nc.tensor.matmul(
                            out=ps,
                            lhsT=wt[kti][:, mt * 128:(mt + 1) * 128],
                            rhs=xt[kti][:, nq * QB:(nq + 1) * QB],
                            start=(kti == 0),
                            stop=(kti == n_dt),
                        )
                    for hl in range(2):
                        h = 2 * mt + hl
                        if evict_eng == "act":
                            nc.scalar.copy(
                                out=dest[h][0:64, nq * QB:(nq + 1) * QB],
                                in_=ps[hl * 64:(hl + 1) * 64, :],
                            )
                        else:
                            nc.vector.tensor_copy(
                                out=dest[h][0:64, nq * QB:(nq + 1) * QB],
                                in_=ps[hl * 64:(hl + 1) * 64, :],
                            )
            for h in range(hpc):
                nc.sync.dma_start(out=dest[h][64:65, :], in_=pad_d[0:1, :])

        qk_proj(xqT, wqT, QT, onesrow_d, "act", [nc.sync, nc.gpsimd, nc.scalar])
        qk_proj(xkT, wkT, KTt, padrow_d, "dve", [nc.gpsimd, nc.scalar, nc.sync])

        # V projection: V[seq, dpc] natural layout, x^T tiles are stationary.
        xt = []
        wt = []
        vengs = [nc.scalar, nc.sync, nc.gpsimd]
        for kti in range(n_dt):
            veng = vengs[kti % 3]
            x_t = xpool.tile([128, seq], bf16, tag="xt")
            veng.dma_start(out=x_t, in_=xvT[kti * 128:(kti + 1) * 128, :])
            w_t = wpool.tile([128, dpc], bf16, tag="wt")
            veng.dma_start(out=w_t, in_=wvT[kti * 128:(kti + 1) * 128, :])
            xt.append(x_t)
            wt.append(w_t)
        x_last = xpool.tile([1, seq], bf16, tag="xlast")
        nc.scalar.dma_start(out=x_last, in_=xvT[d:d + 1, :])
        w_last = wpool.tile([1, dpc], bf16, tag="wlast")
        nc.scalar.dma_start(out=w_last, in_=wvT[d:d + 1, :])
        xt.append(x_last)
        wt.append(w_last)

        for mt in range(n_kt):
            ps = psD.tile([128, dpc], fp32, tag="proj")
            for kti in range(n_dt + 1):
                nc.tensor.matmul(
                    out=ps,
                    lhsT=xt[kti][:, mt * 128:(mt + 1) * 128],
                    rhs=wt[kti][:, :],
                    start=(kti == 0),
                    stop=(kti == n_dt),
                )
            nc.vector.tensor_copy(
                out=V_sb[mt][:, :, 0:64],
                in_=ps.rearrange("p (h e) -> p h e", h=hpc),
            )
            nc.vector.memset(V_sb[mt][:, :, 64:65], 1.0)

        # ================= phase B: attention (+ interleaved out-proj/RS) ==
        wo_sb = [persist.tile([128, d], bf16, tag=f"wo{t}", name=f"wo{t}") for t in range(n_mt)]
        for t in range(n_mt):
            nc.sync.dma_start(out=wo_sb[t], in_=woT[t * 128:(t + 1) * 128, :])

        for qb in range(n_qb):
            for h in range(hpc):
                last_kt = sub * qb + sub - 1
                pts = []
                for kti in range(last_kt + 1):
                    o = max(0, 128 * (kti - sub * qb))
                    s_ps = psA.tile([128, QB], fp32, tag="s")
                    nc.tensor.matmul(
                        out=s_ps[:, o:QB],
                        lhsT=KTt[h][:, kti * KT:(kti + 1) * KT],
                        rhs=QT[h][:, qb * QB + o:(qb + 1) * QB],
                        start=True,
                        stop=True,
                    )
                    pt = ppool.tile([128, QB], bf16, tag="pt")
                    nc.scalar.activation(
                        out=pt[:, o:QB],
                        in_=s_ps[:, o:QB],
                        func=mybir.ActivationFunctionType.Exp,
                    )
                    if kti >= sub * qb:
                        v = kti - sub * qb
                        nc.vector.tensor_mul(
                            out=pt[:, o:QB],
                            in0=pt[:, o:QB],
                            in1=masks_sb[:, v, o:QB],
                        )
                    pts.append(pt)
                # P@V with P^T stationary: full-width 128-contraction
                # matmuls; ctx lands q-major so normalization is a cheap
                # per-partition scalar multiply.  One PSUM accumulation
                # group (bank) per 128-q chunk.
                t, r = divmod(h * 64, 128)
                for qs in range(sub):
                    ctx_ps = psB.tile([128, 65], fp32, tag="ctx")
                    for kti in range(sub * qb + qs + 1):
                        nc.tensor.matmul(
                            out=ctx_ps,
                            lhsT=pts[kti][:, qs * 128:(qs + 1) * 128],
                            rhs=V_sb[kti][:, h, :],
                            start=(kti == 0),
                            stop=(kti == sub * qb + qs),
                        )
                    rcp = spool.tile([128, 1], fp32, tag="rcp")
                    nc.vector.reciprocal(out=rcp, in_=ctx_ps[:, 64:65])
                    cq = cqpool.tile([128, 64], bf16, tag="cq")
                    nc.vector.tensor_scalar(
                        out=cq,
                        in0=ctx_ps[:, 0:64],
                        scalar1=rcp,
                        scalar2=None,
                        op0=mybir.AluOpType.mult,
                    )
                    tr_ps = psT.tile([64, 128], bf16, tag="tr")
                    nc.tensor.transpose(out=tr_ps, in_=cq, identity=ident_sb)
                    nc.vector.tensor_copy(
                        out=ctxT[t][r:r + 64,
                                    qb * QB + qs * 128:qb * QB + (qs + 1) * 128],
                        in_=tr_ps,
                    )

            # out-proj for this q block; RS chunk overlaps the next q block.
            for mt in range(sub * qb, sub * qb + sub):
                po = opool.tile([128, d], bf16, tag="po")
                for oc in range(d // QB):
                    ps = psD.tile([128, QB], fp32, tag="proj")
                    for t in range(n_mt):
                        nc.tensor.matmul(
                            out=ps,
                            lhsT=ctxT[t][:, mt * 128:(mt + 1) * 128],
                            rhs=wo_sb[t][:, oc * QB:(oc + 1) * QB],
                            start=(t == 0),
                            stop=(t == n_mt - 1),
                        )
                    nc.vector.tensor_add(
                        out=po[:, oc * QB:(oc + 1) * QB],
                        in0=ps,
                        in1=bo4_sb[:, oc * QB:(oc + 1) * QB],
                    )
                nc.gpsimd.dma_start(out=partial_dram[mt * 128:(mt + 1) * 128, :], in_=po)

            # ReduceScatter a pair of q blocks across the 4-core group:
            # group rank j receives reduced rows 2*QB*c + 256*j (chunk-major
            # output layout; the host reassembles).
            if (qb + 1) % rsp == 0:
                c = qb // rsp
                if last_split and qb == n_qb - 1:
                    # split the final chunk so the exposed tail collective
                    # is half-size (it cannot overlap anything)
                    for hf in range(2):
                        r0 = qb * QB + hf * 256
                        nc.gpsimd.collective_compute(
                            "ReduceScatter",
                            mybir.AluOpType.add,
                            replica_groups=groups,
                            ins=[partial_dram[r0:r0 + 256, :]],
                            outs=[rs_l[hf, :, :]],
                        )
                        osb = opool.tile([128, d], bf16, tag="osb")
                        nc.sync.dma_start(out=osb[0:64, :], in_=rs_l[hf, :, :])
                        nc.sync.dma_start(
                            out=out_ext[c, hf * 64:(hf + 1) * 64, :], in_=osb[0:64, :])
                else:
                    nc.gpsimd.collective_compute(
                        "ReduceScatter",
                        mybir.AluOpType.add,
                        replica_groups=groups,
                        ins=[partial_dram[(qb + 1 - rsp) * QB:(qb + 1) * QB, :]],
                        outs=[rs_out[c, :, :]],
                    )
                    for j2 in range(rsp):
                        osb = opool.tile([128, d], bf16, tag="osb")
                        nc.sync.dma_start(out=osb, in_=rs_out[c, j2 * 128:(j2 + 1) * 128, :])
                        nc.sync.dma_start(out=out_ext[c, j2 * 128:(j2 + 1) * 128, :], in_=osb)

    # Legalize for TRN2 (max 1 sync wait per instruction, matmul waits
    # moved onto ldweights, nop fusion, register allocation).
    nc.compile()
    return nc


def prep_core_inputs(inputs, core, seq=SEQ, d=D, hpc=HPC):
    """Host-side shard/layout prep for one core.  Pure layout + dtype work."""
    b, g = divmod(core, G)
    dpc = hpc * DH
    sl = slice(g * dpc, (g + 1) * dpc)
    sub = QB // KT

    def aug_xT(x):
        out = np.empty((d + 1, seq), dtype=BF16)
        out[:d] = np.ascontiguousarray(x[b].T).astype(BF16)
        out[d] = BF16(1.0)
        return out

    def aug_wT(w, bias, scale=1.0):
        out = np.empty((d + 1, dpc), dtype=np.float32)
        out[:d] = w[sl, :].T
        out[d] = bias[sl]
        return (out * scale).astype(BF16)

    q, k, v = inputs["q_input"], inputs["k_input"], inputs["v_input"]
    kpm = np.asarray(inputs["key_padding_mask"])

    ktile = np.arange(KT)[:, None]
    qcol = np.arange(QB)[None, :]
    masks = np.stack(
        [(ktile + 128 * vv <= qcol) for vv in range(sub)], axis=1
    ).astype(BF16)  # [KT, sub, QB], 1.0 where key <= query

    return {
        "xqT": aug_xT(np.asarray(q)),
        "xkT": aug_xT(np.asarray(k)),
        "xvT": aug_xT(np.asarray(v)),
        "wqT": aug_wT(np.asarray(inputs["wq"]), np.asarray(inputs["bq"]),
                      scale=1.0 / np.sqrt(DH)),
        "wkT": aug_wT(np.asarray(inputs["wk"]), np.asarray(inputs["bk"])),
        "wvT": aug_wT(np.asarray(inputs["wv"]), np.asarray(inputs["bv"])),
        "woT": np.ascontiguousarray(np.asarray(inputs["wo"]).T[sl, :]).astype(BF16),
        "masks": masks,
        "ident": np.eye(128, dtype=BF16),
        "onesrow": np.ones((1, seq), dtype=BF16),
        "padrow": (NEG * kpm[b].astype(np.float32))[None, :].astype(BF16),
        "bo4": np.tile(np.asarray(inputs["bo"]).astype(np.float32) / G, (128, 1)),
    }


def assemble_output(core_outs, seq=SEQ, d=D):
    """core_outs[4b+j] has shape [n_ch, rsp*128, d]: chunk c holds reduced
    output rows rsp*QB*c + rsp*128*j of batch b."""
    out = np.empty((B, seq, d), dtype=np.float32)
    n_qb = seq // QB
    rsp = 1
    n_ch = n_qb // rsp
    rows = rsp * 128
    last_split = False
    for core in range(NCORES):
        b, j = divmod(core, G)
        co = np.asarray(core_outs[core]).astype(np.float32).reshape(n_ch, rows, d)
        for c in range(n_ch):
            if last_split and c == n_ch - 1:
                for hf in range(2):
                    r0 = QB * c + hf * 256 + 64 * j
                    out[b, r0:r0 + 64, :] = co[c, hf * 64:(hf + 1) * 64]
            else:
                r0 = rsp * QB * c + rows * j
                out[b, r0:r0 + rows, :] = co[c]
    return out


_CACHED_NC = None


def _get_nc():
    global _CACHED_NC
    if _CACHED_NC is None:
        _CACHED_NC = build_program()
    return _CACHED_NC


def kernel(**inputs) -> np.ndarray:
    nc = _get_nc()
    in_maps = [prep_core_inputs(inputs, core) for core in range(NCORES)]
    res = run_bass_kernel_spmd(nc, in_maps, core_ids=list(range(NCORES)))
    return assemble_output([res.results[c]["out"] for c in range(NCORES)])


if __name__ == "__main__":
    nc = build_program()
    print("program built ok")


# revision 41
# speedup vs baseline: 1.0139x; 1.0139x over previous
"""Trainium2 Bass kernel for nn_AttentionUnit (multi-head attention block).

Reference math (B=2, S=2048, D=1024, H=16 heads, d_head=64, fp32):
    Q = q @ wq.T + bq ; K = k @ wk.T + bk ; V = v @ wv.T + bv
    S = QK^T / 8  (per head), causal mask + key-padding mask
    out = softmax(S) @ V  -> concat heads -> @ wo.T + bo

Sharding (8 cores): data-parallel over batch (2 groups of 4 cores),
tensor-parallel over heads (4 heads/core).  Column-parallel QKV,
row-parallel wo with per-q-block in-group ReduceScatter(add) of the
partial outputs, interleaved with the attention loop so the collectives
overlap compute; the host reassembles the chunk-major outputs.

Device-side layout choices:
  - All activations/weights enter as bf16 (fp32 accumulation in PSUM);
    host pre-transposes x to x^T[D, S] and appends a ones-row so
    projection biases ride the matmul contraction ([D+1] rows).
  - Q^T/K^T are kept head-major [65, S]: row 64 of Q^T is ones, row 64
    of K^T is -30000*key_padding_mask, so padded keys get exp()==0 via
    the same contraction trick.
  - Scores are computed transposed (S^T[k, q], keys on partitions) so
    exp(S^T) = P^T feeds P@V directly as the stationary operand;
    softmax max-subtraction is skipped (scores are O(1) here).  The
    row-sum rides a ones-column appended to V; ctx lands q-major so the
    division is a per-partition reciprocal+scale, then a PE transpose
    puts ctx^T in place for the row-parallel output projection.
  - Causal masking is exact: k-tiles are sliced to the valid q range,
    and the 4 diagonal-band tiles per 512-wide q block are multiplied
    by constant 0/1 masks after exp().
  - Input DMAs round-robin across the three DMA-capable queues
    (sync/scalar/gpsimd); collectives own the gpsimd queue in steady
    state.
"""

import os
import sys
from contextlib import ExitStack

import numpy as np

try:
    import concourse.bass as bass
except ImportError:  # harness containers keep the repo at /opt/trn_rl_repo
    for _p in ("/opt/trn_rl_repo", "/root/.axon_site/_ro/trn_rl_repo"):
        if os.path.isdir(_p) and _p not in sys.path:
            sys.path.insert(0, _p)
    import concourse.bass as bass

from concourse import bacc

import ml_dtypes
import concourse.mybir as mybir
import concourse.tile as tile
from concourse.bass_utils import run_bass_kernel_spmd

BF16 = ml_dtypes.bfloat16

B = 2
SEQ = 2048
D = 1024
H = 16
DH = 64
NCORES = 8
G = 4            # tensor-parallel group size (cores per batch)
HPC = H // G     # heads per core
DPC = HPC * DH   # head dims per core (256)
QB = 512         # q block width
KT = 128         # k tile height
NEG = -30000.0   # "minus infinity" for masking (exp() underflows to 0)


def build_program(seq=SEQ, d=D, hpc=HPC, trace_friendly=False):
    """Emit the SPMD program (identical on all 8 cores)."""
    fp32 = mybir.dt.float32
    bf16 = mybir.dt.bfloat16
    dpc = hpc * DH
    n_qb = seq // QB
    n_kt = seq // KT
    n_dt = d // 128          # 128-row tiles of the model dim
    n_mt = dpc // 128        # 128-col tiles of the per-core head dims
    sub = QB // KT           # k-tiles per q block on the diagonal (4)
    rsp = 1                           # q blocks per ReduceScatter chunk
    out_rows = seq // G

    nc = bacc.Bacc(num_devices=NCORES)

    xqT = nc.declare_dram_parameter("xqT", [d + 1, seq], bf16, False)
    xkT = nc.declare_dram_parameter("xkT", [d + 1, seq], bf16, False)
    xvT = nc.declare_dram_parameter("xvT", [d + 1, seq], bf16, False)
    wqT = nc.declare_dram_parameter("wqT", [d + 1, dpc], bf16, False)
    wkT = nc.declare_dram_parameter("wkT", [d + 1, dpc], bf16, False)
    wvT = nc.declare_dram_parameter("wvT", [d + 1, dpc], bf16, False)
    woT = nc.declare_dram_parameter("woT", [dpc, d], bf16, False)
    masks_d = nc.declare_dram_parameter("masks", [KT, sub, QB], bf16, False)
    ident_d = nc.declare_dram_parameter("ident", [128, 128], bf16, False)
    onesrow_d = nc.declare_dram_parameter("onesrow", [1, seq], bf16, False)
    padrow_d = nc.declare_dram_parameter("padrow", [1, seq], bf16, False)
    bo4_d = nc.declare_dram_parameter("bo4", [128, d], fp32, False)
    out_ext = nc.declare_dram_parameter("out", [n_qb // rsp, rsp * 128, d], bf16, isOutput=True)

    partial_dram = nc.dram_tensor("partial", [seq, d], bf16)
    rs_out = nc.dram_tensor("rs_out", [n_qb // rsp, rsp * 128, d], bf16)
    last_split = False  # two tail collectives serialize: worse
    rs_l = nc.dram_tensor("rs_l", [2, 64, d], bf16) if last_split else None

    groups = [[0, 1, 2, 3], [4, 5, 6, 7]]

    with ExitStack() as ctx:
        tc = ctx.enter_context(tile.TileContext(nc, num_cores=NCORES))

        xpool = ctx.enter_context(tc.tile_pool(name="xp", bufs=24))
        wpool = ctx.enter_context(tc.tile_pool(name="wp", bufs=12))
        persist = ctx.enter_context(tc.tile_pool(name="persist", bufs=1))
        vpool = ctx.enter_context(tc.tile_pool(name="vp", bufs=1))
        ppool = ctx.enter_context(tc.tile_pool(name="pp", bufs=20))
        spool = ctx.enter_context(tc.tile_pool(name="sp", bufs=4))
        opool = ctx.enter_context(tc.tile_pool(name="op", bufs=4))
        cqpool = ctx.enter_context(tc.tile_pool(name="cq", bufs=3))
        psA = ctx.enter_context(tc.tile_pool(name="psA", bufs=3, space="PSUM"))
        psB = ctx.enter_context(tc.tile_pool(name="psB", bufs=2, space="PSUM"))
        psT = ctx.enter_context(tc.tile_pool(name="psT", bufs=1, space="PSUM"))
        psD = ctx.enter_context(tc.tile_pool(name="psD", bufs=2, space="PSUM"))

        # ---- constants ----
        masks_sb = persist.tile([KT, sub, QB], bf16, tag="masks")
        nc.gpsimd.dma_start(out=masks_sb, in_=masks_d[:, :, :])
        bo4_sb = persist.tile([128, d], fp32, tag="bo4")
        nc.gpsimd.dma_start(out=bo4_sb, in_=bo4_d[:, :])
        ident_sb = persist.tile([128, 128], bf16, tag="ident")
        nc.gpsimd.dma_start(out=ident_sb, in_=ident_d[:, :])

        # ---- persistent activation tiles ----
        QT = [persist.tile([65, seq], bf16, tag=f"QT{h}", name=f"QT{h}") for h in range(hpc)]
        KTt = [persist.tile([65, seq], bf16, tag=f"KT{h}", name=f"KT{h}") for h in range(hpc)]
        V_sb = [vpool.tile([128, hpc, 65], bf16, tag=f"V{m}", name=f"V{m}") for m in range(n_kt)]
        ctxT = [persist.tile([128, seq], bf16, tag=f"ctxT{t}", name=f"ctxT{t}") for t in range(n_mt)]

        # ================= phase A: projections =================
        n_half = 2 if seq >= 2 * QB else 1
        half = seq // n_half

        def qk_proj(xT_d, wT_d, dest, pad_d, evict_eng, dma_engs):
            xt = []
            wt = []
            for kti in range(n_dt):
                eng = dma_engs[kti % len(dma_engs)]
                halves = []
                for hh in range(n_half):
                    x_t = xpool.tile([128, half], bf16, tag="xt",
                                     name=f"xt{kti}_{hh}")
                    eng.dma_start(
                        out=x_t,
                        in_=xT_d[kti * 128:(kti + 1) * 128,
                                 hh * half:(hh + 1) * half])
                    halves.append(x_t)
                w_t = wpool.tile([128, dpc], bf16, tag="wt")
                eng.dma_start(out=w_t, in_=wT_d[kti * 128:(kti + 1) * 128, :])
                xt.append(halves)
                wt.append(w_t)
            x_last = xpool.tile([1, seq], bf16, tag="xlast", bufs=3)
            dma_engs[0].dma_start(out=x_last, in_=xT_d[d:d + 1, :])
            w_last = wpool.tile([1, dpc], bf16, tag="wlast", bufs=3)
            dma_engs[0].dma_start(out=w_last, in_=wT_d[d:d + 1, :])
            xt.append([x_last] * n_half)
            wt.append(w_last)

            for mt in range(n_mt):
                for nq in range(n_qb):
                    hh, cc = divmod(nq * QB, half)
                    for kti in range(n_dt + 1):
                        if kti == n_dt:
                            rhs = x_last[:, nq * QB:(nq + 1) * QB]
                        else:
                            rhs = xt[kti][hh][:, cc:cc + QB]
                        if kti == 0:
                            ps = psD.tile([128, QB], fp32, tag="proj")
                        nc.tensor.matmul(
                            out=ps,
                            lhsT=wt[kti][:, mt * 128:(mt + 1) * 128],
                            rhs=rhs,
                            start=(kti == 0),
                            stop=(kti == n_dt),
                        )
                    for hl in range(2):
                        h = 2 * mt + hl
                        if evict_eng == "act":
                            nc.scalar.copy(
                                out=dest[h][0:64, nq * QB:(nq + 1) * QB],
                                in_=ps[hl * 64:(hl + 1) * 64, :],
                            )
                        else:
                            nc.vector.tensor_copy(
                                out=dest[h][0:64, nq * QB:(nq + 1) * QB],
                                in_=ps[hl * 64:(hl + 1) * 64, :],
                            )
            for h in range(hpc):
                nc.sync.dma_start(out=dest[h][64:65, :], in_=pad_d[0:1, :])

        qk_proj(xqT, wqT, QT, onesrow_d, "act", [nc.sync, nc.gpsimd, nc.scalar])
        qk_proj(xkT, wkT, KTt, padrow_d, "dve", [nc.gpsimd, nc.scalar, nc.sync])

        # V projection: V[seq, dpc] natural layout, x^T tiles are stationary.
        xt = []
        wt = []
        vengs = [nc.scalar, nc.sync, nc.gpsimd]
        for kti in range(n_dt):
            veng = vengs[kti % 3]
            halves = []
            for hh in range(n_half):
                x_t = xpool.tile([128, half], bf16, tag="xt",
                                 name=f"xvt{kti}_{hh}")
                veng.dma_start(
                    out=x_t,
                    in_=xvT[kti * 128:(kti + 1) * 128, hh * half:(hh + 1) * half])
                halves.append(x_t)
            w_t = wpool.tile([128, dpc], bf16, tag="wt")
            veng.dma_start(out=w_t, in_=wvT[kti * 128:(kti + 1) * 128, :])
            xt.append(halves)
            wt.append(w_t)
        x_last = xpool.tile([1, seq], bf16, tag="xlast", bufs=3)
        nc.scalar.dma_start(out=x_last, in_=xvT[d:d + 1, :])
        w_last = wpool.tile([1, dpc], bf16, tag="wlast", bufs=3)
        nc.scalar.dma_start(out=w_last, in_=wvT[d:d + 1, :])
        xt.append(x_last)
        wt.append(w_last)

        for mt in range(n_kt):
            hh, cc = divmod(mt * 128, half)
            ps = psD.tile([128, dpc], fp32, tag="proj")
            for kti in range(n_dt + 1):
                if kti == n_dt:
                    lhsT = x_last[:, mt * 128:(mt + 1) * 128]
                else:
                    lhsT = xt[kti][hh][:, cc:cc + 128]
                nc.tensor.matmul(
                    out=ps,
                    lhsT=lhsT,
                    rhs=wt[kti][:, :],
                    start=(kti == 0),
                    stop=(kti == n_dt),
                )
            nc.vector.tensor_copy(
                out=V_sb[mt][:, :, 0:64],
                in_=ps.rearrange("p (h e) -> p h e", h=hpc),
            )
            nc.vector.memset(V_sb[mt][:, :, 64:65], 1.0)

        # ================= phase B: attention (+ interleaved out-proj/RS) ==
        wo_sb = [persist.tile([128, d], bf16, tag=f"wo{t}", name=f"wo{t}") for t in range(n_mt)]
        for t in range(n_mt):
            nc.sync.dma_start(out=wo_sb[t], in_=woT[t * 128:(t + 1) * 128, :])

        for qb in range(n_qb):
            for h in range(hpc):
                last_kt = sub * qb + sub - 1
                pts = []
                for kti in range(last_kt + 1):
                    o = max(0, 128 * (kti - sub * qb))
                    s_ps = psA.tile([128, QB], fp32, tag="s")
                    nc.tensor.matmul(
                        out=s_ps[:, o:QB],
                        lhsT=KTt[h][:, kti * KT:(kti + 1) * KT],
                        rhs=QT[h][:, qb * QB + o:(qb + 1) * QB],
                        start=True,
                        stop=True,
                    )
                    pt = ppool.tile([128, QB], bf16, tag="pt")
                    nc.scalar.activation(
                        out=pt[:, o:QB],
                        in_=s_ps[:, o:QB],
                        func=mybir.ActivationFunctionType.Exp,
                    )
                    if kti >= sub * qb:
                        v = kti - sub * qb
                        nc.vector.tensor_mul(
                            out=pt[:, o:QB],
                            in0=pt[:, o:QB],
                            in1=masks_sb[:, v, o:QB],
                        )
                    pts.append(pt)
                # P@V with P^T stationary: full-width 128-contraction
                # matmuls; ctx lands q-major so normalization is a cheap
                # per-partition scalar multiply.  One PSUM accumulation
                # group (bank) per 128-q chunk.
                t, r = divmod(h * 64, 128)
                for qs in range(sub):
                    ctx_ps = psB.tile([128, 65], fp32, tag="ctx")
                    for kti in range(sub * qb + qs + 1):
                        nc.tensor.matmul(
                            out=ctx_ps,
                            lhsT=pts[kti][:, qs * 128:(qs + 1) * 128],
                            rhs=V_sb[kti][:, h, :],
                            start=(kti == 0),
                            stop=(kti == sub * qb + qs),
                        )
                    rcp = spool.tile([128, 1], fp32, tag="rcp")
                    nc.vector.reciprocal(out=rcp, in_=ctx_ps[:, 64:65])
                    cq = cqpool.tile([128, 64], bf16, tag="cq")
                    nc.vector.tensor_scalar(
                        out=cq,
                        in0=ctx_ps[:, 0:64],
                        scalar1=rcp,
                        scalar2=None,
                        op0=mybir.AluOpType.mult,
                    )
                    tr_ps = psT.tile([64, 128], bf16, tag="tr")
                    nc.tensor.transpose(out=tr_ps, in_=cq, identity=ident_sb)
                    nc.vector.tensor_copy(
                        out=ctxT[t][r:r + 64,
                                    qb * QB + qs * 128:qb * QB + (qs + 1) * 128],
                        in_=tr_ps,
                    )

            # out-proj for this q block; RS chunk overlaps the next q block.
            for mt in range(sub * qb, sub * qb + sub):
                po = opool.tile([128, d], bf16, tag="po")
                for oc in range(d // QB):
                    ps = psD.tile([128, QB], fp32, tag="proj")
                    for t in range(n_mt):
                        nc.tensor.matmul(
                            out=ps,
                            lhsT=ctxT[t][:, mt * 128:(mt + 1) * 128],
                            rhs=wo_sb[t][:, oc * QB:(oc + 1) * QB],
                            start=(t == 0),
                            stop=(t == n_mt - 1),
                        )
                    nc.vector.tensor_add(
                        out=po[:, oc * QB:(oc + 1) * QB],
                        in0=ps,
                        in1=bo4_sb[:, oc * QB:(oc + 1) * QB],
                    )
                nc.gpsimd.dma_start(out=partial_dram[mt * 128:(mt + 1) * 128, :], in_=po)

            # ReduceScatter a pair of q blocks across the 4-core group:
            # group rank j receives reduced rows 2*QB*c + 256*j (chunk-major
            # output layout; the host reassembles).
            if (qb + 1) % rsp == 0:
                c = qb // rsp
                if last_split and qb == n_qb - 1:
                    # split the final chunk so the exposed tail collective
                    # is half-size (it cannot overlap anything)
                    for hf in range(2):
                        r0 = qb * QB + hf * 256
                        nc.gpsimd.collective_compute(
                            "ReduceScatter",
                            mybir.AluOpType.add,
                            replica_groups=groups,
                            ins=[partial_dram[r0:r0 + 256, :]],
                            outs=[rs_l[hf, :, :]],
                        )
                        osb = opool.tile([128, d], bf16, tag="osb")
                        nc.sync.dma_start(out=osb[0:64, :], in_=rs_l[hf, :, :])
                        nc.sync.dma_start(
                            out=out_ext[c, hf * 64:(hf + 1) * 64, :], in_=osb[0:64, :])
                else:
                    nc.gpsimd.collective_compute(
                        "ReduceScatter",
                        mybir.AluOpType.add,
                        replica_groups=groups,
                        ins=[partial_dram[(qb + 1 - rsp) * QB:(qb + 1) * QB, :]],
                        outs=[rs_out[c, :, :]],
                    )
                    for j2 in range(rsp):
                        osb = opool.tile([128, d], bf16, tag="osb")
                        nc.sync.dma_start(out=osb, in_=rs_out[c, j2 * 128:(j2 + 1) * 128, :])
                        nc.sync.dma_start(out=out_ext[c, j2 * 128:(j2 + 1) * 128, :], in_=osb)

    # Legalize for TRN2 (max 1 sync wait per instruction, matmul waits
    # moved onto ldweights, nop fusion, register allocation).
    nc.compile()
    return nc


def prep_core_inputs(inputs, core, seq=SEQ, d=D, hpc=HPC):
    """Host-side shard/layout prep for one core.  Pure layout + dtype work."""
    b, g = divmod(core, G)
    dpc = hpc * DH
    sl = slice(g * dpc, (g + 1) * dpc)
    sub = QB // KT

    def aug_xT(x):
        out = np.empty((d + 1, seq), dtype=BF16)
        out[:d] = np.ascontiguousarray(x[b].T).astype(BF16)
        out[d] = BF16(1.0)
        return out

    def aug_wT(w, bias, scale=1.0):
        out = np.empty((d + 1, dpc), dtype=np.float32)
        out[:d] = w[sl, :].T
        out[d] = bias[sl]
        return (out * scale).astype(BF16)

    q, k, v = inputs["q_input"], inputs["k_input"], inputs["v_input"]
    kpm = np.asarray(inputs["key_padding_mask"])

    ktile = np.arange(KT)[:, None]
    qcol = np.arange(QB)[None, :]
    masks = np.stack(
        [(ktile + 128 * vv <= qcol) for vv in range(sub)], axis=1
    ).astype(BF16)  # [KT, sub, QB], 1.0 where key <= query

    return {
        "xqT": aug_xT(np.asarray(q)),
        "xkT": aug_xT(np.asarray(k)),
        "xvT": aug_xT(np.asarray(v)),
        "wqT": aug_wT(np.asarray(inputs["wq"]), np.asarray(inputs["bq"]),
                      scale=1.0 / np.sqrt(DH)),
        "wkT": aug_wT(np.asarray(inputs["wk"]), np.asarray(inputs["bk"])),
        "wvT": aug_wT(np.asarray(inputs["wv"]), np.asarray(inputs["bv"])),
        "woT": np.ascontiguousarray(np.asarray(inputs["wo"]).T[sl, :]).astype(BF16),
        "masks": masks,
        "ident": np.eye(128, dtype=BF16),
        "onesrow": np.ones((1, seq), dtype=BF16),
        "padrow": (NEG * kpm[b].astype(np.float32))[None, :].astype(BF16),
        "bo4": np.tile(np.asarray(inputs["bo"]).astype(np.float32) / G, (128, 1)),
    }


def assemble_output(core_outs, seq=SEQ, d=D):
    """core_outs[4b+j] has shape [n_ch, rsp*128, d]: chunk c holds reduced
    output rows rsp*QB*c + rsp*128*j of batch b."""
    out = np.empty((B, seq, d), dtype=np.float32)
    n_qb = seq // QB
    rsp = 1
    n_ch = n_qb // rsp
    rows = rsp * 128
    last_split = False
    for core in range(NCORES):
        b, j = divmod(core, G)
        co = np.asarray(core_outs[core]).astype(np.float32).reshape(n_ch, rows, d)
        for c in range(n_ch):
            if last_split and c == n_ch - 1:
                for hf in range(2):
                    r0 = QB * c + hf * 256 + 64 * j
                    out[b, r0:r0 + 64, :] = co[c, hf * 64:(hf + 1) * 64]
            else:
                r0 = rsp * QB * c + rows * j
                out[b, r0:r0 + rows, :] = co[c]
    return out


_CACHED_NC = None


def _get_nc():
    global _CACHED_NC
    if _CACHED_NC is None:
        _CACHED_NC = build_program()
    return _CACHED_NC


def kernel(**inputs) -> np.ndarray:
    nc = _get_nc()
    in_maps = [prep_core_inputs(inputs, core) for core in range(NCORES)]
    res = run_bass_kernel_spmd(nc, in_maps, core_ids=list(range(NCORES)))
    return assemble_output([res.results[c]["out"] for c in range(NCORES)])


if __name__ == "__main__":
    nc = build_program()
    print("program built ok")


# revision 42
# speedup vs baseline: 1.0171x; 1.0032x over previous
"""Trainium2 Bass kernel for nn_AttentionUnit (multi-head attention block).

Reference math (B=2, S=2048, D=1024, H=16 heads, d_head=64, fp32):
    Q = q @ wq.T + bq ; K = k @ wk.T + bk ; V = v @ wv.T + bv
    S = QK^T / 8  (per head), causal mask + key-padding mask
    out = softmax(S) @ V  -> concat heads -> @ wo.T + bo

Sharding (8 cores): data-parallel over batch (2 groups of 4 cores),
tensor-parallel over heads (4 heads/core).  Column-parallel QKV,
row-parallel wo with per-q-block in-group ReduceScatter(add) of the
partial outputs, interleaved with the attention loop so the collectives
overlap compute; the host reassembles the chunk-major outputs.

Device-side layout choices:
  - All activations/weights enter as bf16 (fp32 accumulation in PSUM);
    host pre-transposes x to x^T[D, S] and appends a ones-row so
    projection biases ride the matmul contraction ([D+1] rows).
  - Q^T/K^T are kept head-major [65, S]: row 64 of Q^T is ones, row 64
    of K^T is -30000*key_padding_mask, so padded keys get exp()==0 via
    the same contraction trick.
  - Scores are computed transposed (S^T[k, q], keys on partitions) so
    exp(S^T) = P^T feeds P@V directly as the stationary operand;
    softmax max-subtraction is skipped (scores are O(1) here).  The
    row-sum rides a ones-column appended to V; ctx lands q-major so the
    division is a per-partition reciprocal+scale, then a PE transpose
    puts ctx^T in place for the row-parallel output projection.
  - Causal masking is exact: k-tiles are sliced to the valid q range,
    and the 4 diagonal-band tiles per 512-wide q block are multiplied
    by constant 0/1 masks after exp().
  - Input DMAs round-robin across the three DMA-capable queues
    (sync/scalar/gpsimd); collectives own the gpsimd queue in steady
    state.
"""

import os
import sys
from contextlib import ExitStack

import numpy as np

try:
    import concourse.bass as bass
except ImportError:  # harness containers keep the repo at /opt/trn_rl_repo
    for _p in ("/opt/trn_rl_repo", "/root/.axon_site/_ro/trn_rl_repo"):
        if os.path.isdir(_p) and _p not in sys.path:
            sys.path.insert(0, _p)
    import concourse.bass as bass

from concourse import bacc

import ml_dtypes
import concourse.mybir as mybir
import concourse.tile as tile
from concourse.bass_utils import run_bass_kernel_spmd

BF16 = ml_dtypes.bfloat16

B = 2
SEQ = 2048
D = 1024
H = 16
DH = 64
NCORES = 8
G = 4            # tensor-parallel group size (cores per batch)
HPC = H // G     # heads per core
DPC = HPC * DH   # head dims per core (256)
QB = 512         # q block width
KT = 128         # k tile height
NEG = -30000.0   # "minus infinity" for masking (exp() underflows to 0)


def build_program(seq=SEQ, d=D, hpc=HPC, trace_friendly=False):
    """Emit the SPMD program (identical on all 8 cores)."""
    fp32 = mybir.dt.float32
    bf16 = mybir.dt.bfloat16
    dpc = hpc * DH
    n_qb = seq // QB
    n_kt = seq // KT
    n_dt = d // 128          # 128-row tiles of the model dim
    n_mt = dpc // 128        # 128-col tiles of the per-core head dims
    sub = QB // KT           # k-tiles per q block on the diagonal (4)
    rsp = 1                           # q blocks per ReduceScatter chunk
    out_rows = seq // G

    nc = bacc.Bacc(num_devices=NCORES)

    xqT = nc.declare_dram_parameter("xqT", [d + 1, seq], bf16, False)
    xkT = nc.declare_dram_parameter("xkT", [d + 1, seq], bf16, False)
    xvT = nc.declare_dram_parameter("xvT", [d + 1, seq], bf16, False)
    wqT = nc.declare_dram_parameter("wqT", [d + 1, dpc], bf16, False)
    wkT = nc.declare_dram_parameter("wkT", [d + 1, dpc], bf16, False)
    wvT = nc.declare_dram_parameter("wvT", [d + 1, dpc], bf16, False)
    woT = nc.declare_dram_parameter("woT", [dpc, d], bf16, False)
    masks_d = nc.declare_dram_parameter("masks", [KT, sub, QB], bf16, False)
    ident_d = nc.declare_dram_parameter("ident", [128, 128], bf16, False)
    onesrow_d = nc.declare_dram_parameter("onesrow", [1, seq], bf16, False)
    padrow_d = nc.declare_dram_parameter("padrow", [1, seq], bf16, False)
    bo4_d = nc.declare_dram_parameter("bo4", [128, d], fp32, False)
    out_ext = nc.declare_dram_parameter("out", [n_qb // rsp, rsp * 128, d], bf16, isOutput=True)

    partial_dram = nc.dram_tensor("partial", [seq, d], bf16)
    rs_out = nc.dram_tensor("rs_out", [n_qb // rsp, rsp * 128, d], bf16)
    last_split = False  # two tail collectives serialize: worse
    rs_l = nc.dram_tensor("rs_l", [2, 64, d], bf16) if last_split else None

    groups = [[0, 1, 2, 3], [4, 5, 6, 7]]

    with ExitStack() as ctx:
        tc = ctx.enter_context(tile.TileContext(nc, num_cores=NCORES))

        xpool = ctx.enter_context(tc.tile_pool(name="xp", bufs=24))
        wpool = ctx.enter_context(tc.tile_pool(name="wp", bufs=12))
        persist = ctx.enter_context(tc.tile_pool(name="persist", bufs=1))
        vpool = ctx.enter_context(tc.tile_pool(name="vp", bufs=1))
        ppool = ctx.enter_context(tc.tile_pool(name="pp", bufs=21))
        spool = ctx.enter_context(tc.tile_pool(name="sp", bufs=8))
        opool = ctx.enter_context(tc.tile_pool(name="op", bufs=4))
        cqpool = ctx.enter_context(tc.tile_pool(name="cq", bufs=6))
        psA = ctx.enter_context(tc.tile_pool(name="psA", bufs=3, space="PSUM"))
        psB = ctx.enter_context(tc.tile_pool(name="psB", bufs=2, space="PSUM"))
        psT = ctx.enter_context(tc.tile_pool(name="psT", bufs=1, space="PSUM"))
        psD = ctx.enter_context(tc.tile_pool(name="psD", bufs=2, space="PSUM"))

        # ---- constants ----
        masks_sb = persist.tile([KT, sub, QB], bf16, tag="masks")
        nc.gpsimd.dma_start(out=masks_sb, in_=masks_d[:, :, :])
        bo4_sb = persist.tile([128, d], fp32, tag="bo4")
        nc.gpsimd.dma_start(out=bo4_sb, in_=bo4_d[:, :])
        ident_sb = persist.tile([128, 128], bf16, tag="ident")
        nc.gpsimd.dma_start(out=ident_sb, in_=ident_d[:, :])

        # ---- persistent activation tiles ----
        QT = [persist.tile([65, seq], bf16, tag=f"QT{h}", name=f"QT{h}") for h in range(hpc)]
        KTt = [persist.tile([65, seq], bf16, tag=f"KT{h}", name=f"KT{h}") for h in range(hpc)]
        V_sb = [vpool.tile([128, hpc, 65], bf16, tag=f"V{m}", name=f"V{m}") for m in range(n_kt)]
        ctxT = [persist.tile([128, seq], bf16, tag=f"ctxT{t}", name=f"ctxT{t}") for t in range(n_mt)]

        # ================= phase A: projections =================
        n_half = 2 if seq >= 2 * QB else 1
        half = seq // n_half

        def qk_proj(xT_d, wT_d, dest, pad_d, evict_eng, dma_engs):
            xt = []
            wt = []
            for kti in range(n_dt):
                eng = dma_engs[kti % len(dma_engs)]
                halves = []
                for hh in range(n_half):
                    x_t = xpool.tile([128, half], bf16, tag="xt",
                                     name=f"xt{kti}_{hh}")
                    eng.dma_start(
                        out=x_t,
                        in_=xT_d[kti * 128:(kti + 1) * 128,
                                 hh * half:(hh + 1) * half])
                    halves.append(x_t)
                w_t = wpool.tile([128, dpc], bf16, tag="wt")
                eng.dma_start(out=w_t, in_=wT_d[kti * 128:(kti + 1) * 128, :])
                xt.append(halves)
                wt.append(w_t)
            x_last = xpool.tile([1, seq], bf16, tag="xlast", bufs=3)
            dma_engs[0].dma_start(out=x_last, in_=xT_d[d:d + 1, :])
            w_last = wpool.tile([1, dpc], bf16, tag="wlast", bufs=3)
            dma_engs[0].dma_start(out=w_last, in_=wT_d[d:d + 1, :])
            xt.append([x_last] * n_half)
            wt.append(w_last)

            for mt in range(n_mt):
                for nq in range(n_qb):
                    hh, cc = divmod(nq * QB, half)
                    for kti in range(n_dt + 1):
                        if kti == n_dt:
                            rhs = x_last[:, nq * QB:(nq + 1) * QB]
                        else:
                            rhs = xt[kti][hh][:, cc:cc + QB]
                        if kti == 0:
                            ps = psD.tile([128, QB], fp32, tag="proj")
                        nc.tensor.matmul(
                            out=ps,
                            lhsT=wt[kti][:, mt * 128:(mt + 1) * 128],
                            rhs=rhs,
                            start=(kti == 0),
                            stop=(kti == n_dt),
                        )
                    for hl in range(2):
                        h = 2 * mt + hl
                        if evict_eng == "act":
                            nc.scalar.copy(
                                out=dest[h][0:64, nq * QB:(nq + 1) * QB],
                                in_=ps[hl * 64:(hl + 1) * 64, :],
                            )
                        else:
                            nc.vector.tensor_copy(
                                out=dest[h][0:64, nq * QB:(nq + 1) * QB],
                                in_=ps[hl * 64:(hl + 1) * 64, :],
                            )
            for h in range(hpc):
                nc.sync.dma_start(out=dest[h][64:65, :], in_=pad_d[0:1, :])

        qk_proj(xqT, wqT, QT, onesrow_d, "act", [nc.sync, nc.gpsimd, nc.scalar])
        qk_proj(xkT, wkT, KTt, padrow_d, "dve", [nc.gpsimd, nc.scalar, nc.sync])

        # V projection: V[seq, dpc] natural layout, x^T tiles are stationary.
        xt = []
        wt = []
        vengs = [nc.scalar, nc.sync, nc.gpsimd]
        for kti in range(n_dt):
            veng = vengs[kti % 3]
            halves = []
            for hh in range(n_half):
                x_t = xpool.tile([128, half], bf16, tag="xt",
                                 name=f"xvt{kti}_{hh}")
                veng.dma_start(
                    out=x_t,
                    in_=xvT[kti * 128:(kti + 1) * 128, hh * half:(hh + 1) * half])
                halves.append(x_t)
            w_t = wpool.tile([128, dpc], bf16, tag="wt")
            veng.dma_start(out=w_t, in_=wvT[kti * 128:(kti + 1) * 128, :])
            xt.append(halves)
            wt.append(w_t)
        x_last = xpool.tile([1, seq], bf16, tag="xlast", bufs=3)
        nc.scalar.dma_start(out=x_last, in_=xvT[d:d + 1, :])
        w_last = wpool.tile([1, dpc], bf16, tag="wlast", bufs=3)
        nc.scalar.dma_start(out=w_last, in_=wvT[d:d + 1, :])
        xt.append(x_last)
        wt.append(w_last)

        for mt in range(n_kt):
            hh, cc = divmod(mt * 128, half)
            ps = psD.tile([128, dpc], fp32, tag="proj")
            for kti in range(n_dt + 1):
                if kti == n_dt:
                    lhsT = x_last[:, mt * 128:(mt + 1) * 128]
                else:
                    lhsT = xt[kti][hh][:, cc:cc + 128]
                nc.tensor.matmul(
                    out=ps,
                    lhsT=lhsT,
                    rhs=wt[kti][:, :],
                    start=(kti == 0),
                    stop=(kti == n_dt),
                )
            nc.vector.tensor_copy(
                out=V_sb[mt][:, :, 0:64],
                in_=ps.rearrange("p (h e) -> p h e", h=hpc),
            )
            nc.vector.memset(V_sb[mt][:, :, 64:65], 1.0)

        # ================= phase B: attention (+ interleaved out-proj/RS) ==
        wo_sb = [persist.tile([128, d], bf16, tag=f"wo{t}", name=f"wo{t}") for t in range(n_mt)]
        for t in range(n_mt):
            nc.sync.dma_start(out=wo_sb[t], in_=woT[t * 128:(t + 1) * 128, :])

        for qb in range(n_qb):
            for h in range(hpc):
                last_kt = sub * qb + sub - 1
                pts = []
                for kti in range(last_kt + 1):
                    o = max(0, 128 * (kti - sub * qb))
                    s_ps = psA.tile([128, QB], fp32, tag="s")
                    nc.tensor.matmul(
                        out=s_ps[:, o:QB],
                        lhsT=KTt[h][:, kti * KT:(kti + 1) * KT],
                        rhs=QT[h][:, qb * QB + o:(qb + 1) * QB],
                        start=True,
                        stop=True,
                    )
                    pt = ppool.tile([128, QB], bf16, tag="pt")
                    nc.scalar.activation(
                        out=pt[:, o:QB],
                        in_=s_ps[:, o:QB],
                        func=mybir.ActivationFunctionType.Exp,
                    )
                    if kti >= sub * qb:
                        v = kti - sub * qb
                        nc.vector.tensor_mul(
                            out=pt[:, o:QB],
                            in0=pt[:, o:QB],
                            in1=masks_sb[:, v, o:QB],
                        )
                    pts.append(pt)
                # P@V with P^T stationary: full-width 128-contraction
                # matmuls; ctx lands q-major so normalization is a cheap
                # per-partition scalar multiply.  One PSUM accumulation
                # group (bank) per 128-q chunk.
                t, r = divmod(h * 64, 128)
                for qs in range(sub):
                    ctx_ps = psB.tile([128, 65], fp32, tag="ctx")
                    for kti in range(sub * qb + qs + 1):
                        nc.tensor.matmul(
                            out=ctx_ps,
                            lhsT=pts[kti][:, qs * 128:(qs + 1) * 128],
                            rhs=V_sb[kti][:, h, :],
                            start=(kti == 0),
                            stop=(kti == sub * qb + qs),
                        )
                    rcp = spool.tile([128, 1], fp32, tag="rcp")
                    nc.vector.reciprocal(out=rcp, in_=ctx_ps[:, 64:65])
                    cq = cqpool.tile([128, 64], bf16, tag="cq")
                    nc.vector.tensor_scalar(
                        out=cq,
                        in0=ctx_ps[:, 0:64],
                        scalar1=rcp,
                        scalar2=None,
                        op0=mybir.AluOpType.mult,
                    )
                    tr_ps = psT.tile([64, 128], bf16, tag="tr")
                    nc.tensor.transpose(out=tr_ps, in_=cq, identity=ident_sb)
                    nc.vector.tensor_copy(
                        out=ctxT[t][r:r + 64,
                                    qb * QB + qs * 128:qb * QB + (qs + 1) * 128],
                        in_=tr_ps,
                    )

            # out-proj for this q block; RS chunk overlaps the next q block.
            for mt in range(sub * qb, sub * qb + sub):
                po = opool.tile([128, d], bf16, tag="po")
                for oc in range(d // QB):
                    ps = psD.tile([128, QB], fp32, tag="proj")
                    for t in range(n_mt):
                        nc.tensor.matmul(
                            out=ps,
                            lhsT=ctxT[t][:, mt * 128:(mt + 1) * 128],
                            rhs=wo_sb[t][:, oc * QB:(oc + 1) * QB],
                            start=(t == 0),
                            stop=(t == n_mt - 1),
                        )
                    nc.vector.tensor_add(
                        out=po[:, oc * QB:(oc + 1) * QB],
                        in0=ps,
                        in1=bo4_sb[:, oc * QB:(oc + 1) * QB],
                    )
                nc.gpsimd.dma_start(out=partial_dram[mt * 128:(mt + 1) * 128, :], in_=po)

            # ReduceScatter a pair of q blocks across the 4-core group:
            # group rank j receives reduced rows 2*QB*c + 256*j (chunk-major
            # output layout; the host reassembles).
            if (qb + 1) % rsp == 0:
                c = qb // rsp
                if last_split and qb == n_qb - 1:
                    # split the final chunk so the exposed tail collective
                    # is half-size (it cannot overlap anything)
                    for hf in range(2):
                        r0 = qb * QB + hf * 256
                        nc.gpsimd.collective_compute(
                            "ReduceScatter",
                            mybir.AluOpType.add,
                            replica_groups=groups,
                            ins=[partial_dram[r0:r0 + 256, :]],
                            outs=[rs_l[hf, :, :]],
                        )
                        osb = opool.tile([128, d], bf16, tag="osb")
                        nc.sync.dma_start(out=osb[0:64, :], in_=rs_l[hf, :, :])
                        nc.sync.dma_start(
                            out=out_ext[c, hf * 64:(hf + 1) * 64, :], in_=osb[0:64, :])
                else:
                    nc.gpsimd.collective_compute(
                        "ReduceScatter",
                        mybir.AluOpType.add,
                        replica_groups=groups,
                        ins=[partial_dram[(qb + 1 - rsp) * QB:(qb + 1) * QB, :]],
                        outs=[rs_out[c, :, :]],
                    )
                    for j2 in range(rsp):
                        osb = opool.tile([128, d], bf16, tag="osb")
                        nc.sync.dma_start(out=osb, in_=rs_out[c, j2 * 128:(j2 + 1) * 128, :])
                        nc.sync.dma_start(out=out_ext[c, j2 * 128:(j2 + 1) * 128, :], in_=osb)

    # Legalize for TRN2 (max 1 sync wait per instruction, matmul waits
    # moved onto ldweights, nop fusion, register allocation).
    nc.compile()
    return nc


def prep_core_inputs(inputs, core, seq=SEQ, d=D, hpc=HPC):
    """Host-side shard/layout prep for one core.  Pure layout + dtype work."""
    b, g = divmod(core, G)
    dpc = hpc * DH
    sl = slice(g * dpc, (g + 1) * dpc)
    sub = QB // KT

    def aug_xT(x):
        out = np.empty((d + 1, seq), dtype=BF16)
        out[:d] = np.ascontiguousarray(x[b].T).astype(BF16)
        out[d] = BF16(1.0)
        return out

    def aug_wT(w, bias, scale=1.0):
        out = np.empty((d + 1, dpc), dtype=np.float32)
        out[:d] = w[sl, :].T
        out[d] = bias[sl]
        return (out * scale).astype(BF16)

    q, k, v = inputs["q_input"], inputs["k_input"], inputs["v_input"]
    kpm = np.asarray(inputs["key_padding_mask"])

    ktile = np.arange(KT)[:, None]
    qcol = np.arange(QB)[None, :]
    masks = np.stack(
        [(ktile + 128 * vv <= qcol) for vv in range(sub)], axis=1
    ).astype(BF16)  # [KT, sub, QB], 1.0 where key <= query

    return {
        "xqT": aug_xT(np.asarray(q)),
        "xkT": aug_xT(np.asarray(k)),
        "xvT": aug_xT(np.asarray(v)),
        "wqT": aug_wT(np.asarray(inputs["wq"]), np.asarray(inputs["bq"]),
                      scale=1.0 / np.sqrt(DH)),
        "wkT": aug_wT(np.asarray(inputs["wk"]), np.asarray(inputs["bk"])),
        "wvT": aug_wT(np.asarray(inputs["wv"]), np.asarray(inputs["bv"])),
        "woT": np.ascontiguousarray(np.asarray(inputs["wo"]).T[sl, :]).astype(BF16),
        "masks": masks,
        "ident": np.eye(128, dtype=BF16),
        "onesrow": np.ones((1, seq), dtype=BF16),
        "padrow": (NEG * kpm[b].astype(np.float32))[None, :].astype(BF16),
        "bo4": np.tile(np.asarray(inputs["bo"]).astype(np.float32) / G, (128, 1)),
    }


def assemble_output(core_outs, seq=SEQ, d=D):
    """core_outs[4b+j] has shape [n_ch, rsp*128, d]: chunk c holds reduced
    output rows rsp*QB*c + rsp*128*j of batch b."""
    out = np.empty((B, seq, d), dtype=np.float32)
    n_qb = seq // QB
    rsp = 1
    n_ch = n_qb // rsp
    rows = rsp * 128
    last_split = False
    for core in range(NCORES):
        b, j = divmod(core, G)
        co = np.asarray(core_outs[core]).astype(np.float32).reshape(n_ch, rows, d)
        for c in range(n_ch):
            if last_split and c == n_ch - 1:
                for hf in range(2):
                    r0 = QB * c + hf * 256 + 64 * j
                    out[b, r0:r0 + 64, :] = co[c, hf * 64:(hf + 1) * 64]
            else:
                r0 = rsp * QB * c + rows * j
                out[b, r0:r0 + rows, :] = co[c]
    return out


_CACHED_NC = None


def _get_nc():
    global _CACHED_NC
    if _CACHED_NC is None:
        _CACHED_NC = build_program()
    return _CACHED_NC


def kernel(**inputs) -> np.ndarray:
    nc = _get_nc()
    in_maps = [prep_core_inputs(inputs, core) for core in range(NCORES)]
    res = run_bass_kernel_spmd(nc, in_maps, core_ids=list(range(NCORES)))
    return assemble_output([res.results[c]["out"] for c in range(NCORES)])


if __name__ == "__main__":
    nc = build_program()
    print("program built ok")
